# revision 38
# baseline (speedup 1.0000x reference)
"""Trainium2 Bass kernel for the AttentionLoop module.

Reference computation (S=2048, B=32, D=1024, E=1024):
    h = tanh(einsum('sbd,ed->sbe', dec + enc, W_fc))
    scores = einsum('sbe,e->bs', h, score_w[:,0])
    attn = softmax(scores, axis=1)          # over seq
    out = einsum('bs,sbd->bd', attn, enc)   # (B, D)

Data-parallel over batch across 8 NeuronCores (4 batches/core), core-local.

Per-core kernel, hybrid-precision, h in [e-part, s-free] orientation:
  - pass-1 matmuls use W chunks as stationary, enc chunks as moving:
    out tile = [128 e, 512 s] PSUM accumulated over d-chunks. The first
    2*NPAIR d-chunks run as fp8(e4m3) DoubleRow matmuls; the rest as
    bf16 matmuls (fp8 on all 8 chunks would breach the 2e-2 rel-err
    budget: quantization noise on 6/8 chunks already contributes
    ~1.8e-2). Host pre-scales: enc*4, W*32 -> psum 128x, folded out in
    the tanh evac.
  - decoder bias decW[b,e] = dec @ W.T is precomputed on the HOST in
    fp32 and rides the ScalarE tanh evac as a per-partition bias.
  - scores: the sw-weighted e-reduction is hierarchical. Per e-chunk
    one DVE scalar_tensor_tensor over the full [128, 2048] h tile:
    g += h * sw_col (per-partition scalar) folds multiply + 8-chunk
    accumulation; the final 128-partition reduce is 4 ones-stationary
    matmuls per batch (2048 PE cols/batch instead of 16384).
  - pass-2 out[b] = p @ enc is split to balance DVE vs PE: d-chunks
    0..5 on the DVE (stt with accum_out over the resident [d,s] bf16
    enc tiles), d-chunks 6,7 on the TensorE against a small [s, 256]
    enc copy (en3p), with p transposed to columns via K=1 one-hot
    matmuls of the broadcast p. Both halves hide under the next
    batch's pass-1. The last batch runs pass-2 fully on the TensorE
    (all-DVE would be exposed at the end), via a full [s, d] enc copy.
  - softmax skips max-subtraction (scores are O(1)); Exp activation
    with accum_out gives the denominator partials for free.
  - DMA: two HWDGE queues carry pass-1-critical loads in consumption
    order (sync: w8/etb67/et8, scalar: et80/wb + pass-2 copies); tiny
    out stores ride the GpSimd SWDGE queue so their wait on the
    pass-2 result never stalls the input queues.
"""

import numpy as np

S, B, D, E = 2048, 32, 1024, 1024
NCORES = 8
BLOC = B // NCORES          # 4 batches per core
P = 128                     # partitions
DC = D // P                 # 8 d-chunks
EC = E // P                 # 8 e-chunks
SB = 512                    # moving free dim (PSUM bank)
NSBLK = S // SB             # 4 s-blocks per batch
NSC = S // P                # 16 s-chunks per batch

NPAIR = 3                   # d-chunk pairs done in fp8 DoubleRow
F8CH = 2 * NPAIR            # d-chunks covered by fp8
NBF = DC - F8CH             # bf16 d-chunks
NP2PE = 2                   # pass-2 d-chunks on TensorE (rest on DVE)
NP2DVE = DC - NP2PE
DP2 = NP2PE * P             # pass-2 PE columns
ENC_SCALE = 4.0             # host pre-scale on fp8 enc
W_SCALE = 32.0              # host pre-scale on fp8 W
PSUM_SCALE = ENC_SCALE * W_SCALE   # bf16 W copy is scaled by this too

_compiled = None


def _build_program():
    import concourse.bacc as bacc
    import concourse.mybir as mybir
    import concourse.tile as tile

    f32 = mybir.dt.float32
    bf16 = mybir.dt.bfloat16
    f8 = mybir.dt.float8e4
    AF = mybir.ActivationFunctionType
    DR = mybir.MatmulPerfMode.DoubleRow
    ALU = mybir.AluOpType

    nc = bacc.Bacc("TRN2", target_bir_lowering=False, debug=False,
                   num_devices=NCORES)

    et8_d = nc.declare_dram_parameter("et8", [F8CH * P, BLOC, S], f8,
                                      isOutput=False)
    etb_d = nc.declare_dram_parameter("etb", [D, BLOC, S], bf16, isOutput=False)
    # weight/bias layouts are pre-permuted on the host so each SBUF
    # partition's data is one contiguous DRAM run (128 fat DMA descriptors
    # instead of 256-1024 thin ones -> cheap HWDGE triggers)
    w8_d = nc.declare_dram_parameter("w8", [P, F8CH * E], f8, isOutput=False)
    wb_d = nc.declare_dram_parameter("wb", [P, NBF * E], bf16, isOutput=False)
    decw_d = nc.declare_dram_parameter("decw", [P, EC * BLOC], f32,
                                       isOutput=False)
    swc_d = nc.declare_dram_parameter("swc", [P, EC], bf16, isOutput=False)
    # [s, d] bf16 enc copies for PE-side pass-2: last 2 d-chunks for b<3,
    # full D for the tail batch
    enp_d = nc.declare_dram_parameter("enp", [BLOC - 1, S, DP2], bf16,
                                      isOutput=False)
    en3_d = nc.declare_dram_parameter("en3", [S, D], bf16, isOutput=False)
    out_d = nc.declare_dram_parameter("out", [BLOC, D], f32, isOutput=True)

    with tile.TileContext(nc) as tc:
        with tc.tile_pool(name="const", bufs=1) as const, \
             tc.tile_pool(name="et8", bufs=2) as et8_pool, \
             tc.tile_pool(name="etb", bufs=2) as etb_pool, \
             tc.tile_pool(name="enp", bufs=2) as enp_pool, \
             tc.tile_pool(name="h", bufs=4) as h_pool, \
             tc.tile_pool(name="g", bufs=2) as g_pool, \
             tc.tile_pool(name="pbc", bufs=2) as pbc_pool, \
             tc.tile_pool(name="pcl", bufs=2) as pcl_pool, \
             tc.tile_pool(name="scr", bufs=2) as scr_pool, \
             tc.tile_pool(name="misc", bufs=2) as misc, \
             tc.tile_pool(name="ph", bufs=3, space="PSUM") as ph_pool, \
             tc.tile_pool(name="psc", bufs=2, space="PSUM") as psc_pool, \
             tc.tile_pool(name="pt", bufs=1, space="PSUM") as pt_pool, \
             tc.tile_pool(name="po", bufs=2, space="PSUM") as po_pool:

            etb_r = etb_d.ap().rearrange("(dc p) b s -> p dc b s", p=P)
            wb_r = wb_d.ap().rearrange("p (dc e) -> p dc e", dc=NBF)
            decw_r = decw_d.ap().rearrange("p (ec b) -> p ec b", ec=EC)
            enp_r = enp_d.ap().rearrange("b (sc p) d -> p b sc d", p=P)
            en3_r = en3_d.ap().rearrange("(sc p) d -> p sc d", p=P)
            out_r = out_d.ap().rearrange("b (dc p) -> p b dc", p=P)
            et8_r = et8_d.ap().rearrange("(c p) b s -> p c b s", p=P)
            w8_r = w8_d.ap().rearrange("p (c e) -> p c e", c=F8CH)

            # ---- startup DMAs, critical-first, spread over 3 queues ----
            decw_col = const.tile([P, EC, BLOC], f32)
            swc_sb = const.tile([P, EC], bf16)
            w8_sb = const.tile([P, F8CH, E], f8)
            wb_sb = const.tile([P, NBF, E], bf16)
            et80 = et8_pool.tile([P, F8CH, S], f8, tag="et8", name="et80")
            etb0 = etb_pool.tile([P, DC, S], bf16, tag="etb", name="etb0")

            for c in range(0, F8CH, 2):
                nc.sync.dma_start(w8_sb[:, c:c + 2, :], w8_r[:, c:c + 2, :])
            for c in range(0, F8CH, 2):
                nc.scalar.dma_start(et80[:, c:c + 2, :],
                                    et8_r[:, c:c + 2, 0, :])
            nc.gpsimd.dma_start(etb0[:, F8CH:DC, :], etb_r[:, F8CH:DC, 0, :])
            nc.sync.dma_start(wb_sb[:], wb_r)
            nc.scalar.dma_start(decw_col[:], decw_r)
            nc.scalar.dma_start(swc_sb[:], swc_d.ap())

            ones_col = const.tile([P, 1], bf16)
            nc.vector.memset(ones_col[:], 1.0)
            e0 = const.tile([P, 1], bf16)
            nc.vector.memset(e0[:], 0.0)
            nc.vector.memset(e0[0:1, :], 1.0)

            for b in range(BLOC):
                last = (b == BLOC - 1)
                if b == 0:
                    etb, et8 = etb0, et80
                else:
                    etb = etb_pool.tile([P, DC, S], bf16, tag="etb",
                                        name=f"etb{b}")
                    et8 = et8_pool.tile([P, F8CH, S], f8, tag="et8",
                                        name=f"et8{b}")
                    for c in range(0, F8CH, 2):
                        nc.sync.dma_start(et8[:, c:c + 2, :],
                                          et8_r[:, c:c + 2, b, :])
                    nc.sync.dma_start(etb[:, F8CH:DC, :],
                                      etb_r[:, F8CH:DC, b, :])
                if not last:
                    enp_sb = enp_pool.tile([P, NSC, DP2], bf16, tag="enp",
                                           name=f"enp{b}")
                    nc.scalar.dma_start(enp_sb[:], enp_r[:, b, :, :])
                if b in (1, 2):
                    # tail batch's full [s, d] enc copy, in halves at the
                    # starts of b1 and b2 on the Scalar queue
                    if b == 1:
                        en3_sb = const.tile([P, NSC, D], bf16)
                    q0 = (b - 1) * 8
                    nc.scalar.dma_start(en3_sb[:, q0:q0 + 4, :],
                                        en3_r[:, q0:q0 + 4, :])
                    nc.scalar.dma_start(en3_sb[:, q0 + 4:q0 + 8, :],
                                        en3_r[:, q0 + 4:q0 + 8, :])

                sc_ps = [psc_pool.tile([1, SB], f32, tag="sc",
                                       name=f"sc{j}")
                         for j in range(NSBLK)]
                g_acc = g_pool.tile([P, NSBLK, SB], bf16, tag="g",
                                    name=f"g{b}")

                for ec in range(EC):
                    ecs = slice(ec * P, (ec + 1) * P)
                    # deferred emission of this batch's DVE-pass-2 bf16 enc
                    # chunks on the GpSimd SWDGE queue: separate semaphore
                    # lanes, so these late-completing loads never couple
                    # into pass-1 consumers' conservative DMA waits
                    if not last and ec in (1, 2, 3):
                        dcp = 2 * (ec - 1)
                        nc.scalar.dma_start(etb[:, dcp:dcp + 2, :],
                                            etb_r[:, dcp:dcp + 2, b, :])
                    h_ec = h_pool.tile([P, NSBLK, SB], bf16, tag="h",
                                       name=f"h{ec}")
                    for sblk in range(NSBLK):
                        ss = slice(sblk * SB, (sblk + 1) * SB)
                        # pipeline the previous s-block's partition-reduce
                        # into this matmul stream (PE never waits on DVE)
                        if ec == EC - 1 and sblk >= 1:
                            nc.tensor.matmul(
                                sc_ps[sblk - 1][:], ones_col[:],
                                g_acc[:, sblk - 1, :],
                                start=True, stop=True)
                        ph = ph_pool.tile([P, SB], f32, tag="ph",
                                          name=f"ph{sblk}")
                        for pr in range(NPAIR):
                            nc.tensor.matmul(
                                ph[:],
                                w8_sb[:, 2 * pr:2 * pr + 2, ecs],
                                et8[:, 2 * pr:2 * pr + 2, ss],
                                start=(pr == 0), stop=False,
                                perf_mode=DR)
                        for j in range(NBF):
                            nc.tensor.matmul(
                                ph[:], wb_sb[:, j, ecs],
                                etb[:, F8CH + j, ss],
                                start=False, stop=(j == NBF - 1))
                        nc.scalar.activation(
                            h_ec[:, sblk, :], ph[:], AF.Tanh,
                            bias=decw_col[:, ec, b:b + 1],
                            scale=1.0 / PSUM_SCALE)
                        # the last e-chunk's accumulation runs per-s-block so
                        # the pipelined partition-reduce matmuls above see
                        # fully-accumulated g for s-blocks 0..2
                        if ec == EC - 1:
                            nc.vector.scalar_tensor_tensor(
                                g_acc[:, sblk, :], h_ec[:, sblk, :],
                                swc_sb[:, ec:ec + 1], g_acc[:, sblk, :],
                                ALU.mult, ALU.add)
                    # one DVE op per e-chunk folds the sw multiply and the
                    # chunk accumulation over the whole [128, 2048] tile
                    if ec == 0:
                        nc.vector.scalar_tensor_tensor(
                            g_acc[:], h_ec[:],
                            swc_sb[:, ec:ec + 1], h_ec[:],
                            ALU.mult, ALU.bypass)
                    elif ec < EC - 1:
                        nc.vector.scalar_tensor_tensor(
                            g_acc[:], h_ec[:],
                            swc_sb[:, ec:ec + 1], g_acc[:],
                            ALU.mult, ALU.add)

                # last s-block's partition-reduce (0..2 were pipelined)
                nc.tensor.matmul(
                    sc_ps[NSBLK - 1][:], ones_col[:],
                    g_acc[:, NSBLK - 1, :], start=True, stop=True)

                # ---- softmax (no max-subtraction; scores are O(1)) ----
                p_row = misc.tile([1, S], bf16, tag="p")
                lp = misc.tile([1, NSBLK], f32, tag="lp")
                p_bc = pbc_pool.tile([P, S], bf16, tag="pbc")
                pcol = pcl_pool.tile([P, NSC], bf16, tag="pcol")
                pct_all = pt_pool.tile([P, NSC], f32, tag="pt")
                npsb = NSC // NSBLK  # p-columns per s-block
                en_pe = en3_sb if last else enp_sb
                wid = D if last else DP2
                if last:
                    # tail: pass-1 is done, reuse the ph psum ring
                    po = [ph_pool.tile([1, SB], f32, tag="ph",
                                       name=f"po{b}_{g}")
                          for g in range(D // SB)]
                else:
                    po = [po_pool.tile([1, DP2], f32, tag="po",
                                       name=f"po{b}_0")]
                for sblk in range(NSBLK):
                    ss = slice(sblk * SB, (sblk + 1) * SB)
                    nc.scalar.activation(
                        p_row[:, ss], sc_ps[sblk][:],
                        AF.Exp, accum_out=lp[:, sblk:sblk + 1])
                    nc.gpsimd.partition_broadcast(p_bc[:, ss], p_row[:, ss])
                    for k in range(npsb):
                        sc = sblk * npsb + k
                        nc.tensor.matmul(
                            pct_all[:, sc:sc + 1],
                            p_bc[:, sc * P:(sc + 1) * P], e0[:],
                            start=True, stop=True,
                            skip_group_check=True)
                    nc.scalar.activation(
                        pcol[:, sblk * npsb:(sblk + 1) * npsb],
                        pct_all[:, sblk * npsb:(sblk + 1) * npsb], AF.Copy)
                    # PE-side pass-2 for this s-block's p-columns
                    for k in range(npsb):
                        sc = sblk * npsb + k
                        for g in range(max(1, wid // SB)):
                            gw = min(SB, wid)
                            nc.tensor.matmul(
                                po[g][:], pcol[:, sc:sc + 1],
                                en_pe[:, sc, g * gw:(g + 1) * gw],
                                start=(sc == 0), stop=(sc == NSC - 1))

                lt = misc.tile([1, 1], f32, tag="lt")
                nc.vector.tensor_reduce(lt[:], lp[:], mybir.AxisListType.X,
                                        mybir.AluOpType.add)
                invl = misc.tile([1, 1], f32, tag="invl")
                nc.vector.reciprocal(invl[:], lt[:])

                if not last:
                    # ---- DVE pass-2 for d-chunks 0..NP2DVE-1 ----
                    invl_bc = misc.tile([P, 1], f32, tag="invlbc")
                    nc.gpsimd.partition_broadcast(invl_bc[:], invl[:])
                    ocol = misc.tile([P, NP2DVE], f32, tag="ocol")
                    scr = scr_pool.tile([P, S], bf16, tag="scr")
                    for dc in range(NP2DVE):
                        nc.vector.scalar_tensor_tensor(
                            scr[:], etb[:, dc, :], 1.0, p_bc[:],
                            ALU.mult, ALU.mult,
                            accum_out=ocol[:, dc:dc + 1])
                    outf = misc.tile([P, NP2DVE], f32, tag="outf")
                    nc.scalar.activation(outf[:], ocol[:], AF.Copy,
                                         scale=invl_bc[:])
                    nc.gpsimd.dma_start(out_r[:, b, 0:NP2DVE], outf[:])
                    # PE-side chunks: scale + store the [1, DP2] row
                    orow = misc.tile([1, DP2], f32, tag="orow")
                    nc.scalar.activation(orow[:], po[0][:], AF.Copy,
                                         scale=invl[:])
                    nc.gpsimd.dma_start(
                        out_d.ap()[b:b + 1, D - DP2:D], orow[:])
                else:
                    out_row = misc.tile([1, D], f32, tag="orow3")
                    for g in range(D // SB):
                        nc.scalar.activation(
                            out_row[:, g * SB:(g + 1) * SB], po[g][:],
                            AF.Copy, scale=invl[:])
                    nc.sync.dma_start(out_d.ap()[b:b + 1, :], out_row[:])

    nc.compile()
    return nc


def _get_program():
    global _compiled
    if _compiled is None:
        _compiled = _build_program()
    return _compiled


def make_in_maps(encoder_states, decoder_state, W_fc, score_w):
    """Shard + lay out + cast full inputs into per-core input maps."""
    import ml_dtypes
    f8 = ml_dtypes.float8_e4m3
    bf16 = ml_dtypes.bfloat16

    enc = np.asarray(encoder_states, dtype=np.float32)
    dec = np.asarray(decoder_state, dtype=np.float32)
    wfc = np.asarray(W_fc, dtype=np.float32)
    sw = np.asarray(score_w, dtype=np.float32)

    w_t = np.ascontiguousarray(wfc.T)                      # (D, E)
    # per-partition-contiguous permutations: [chunk, p, e] -> [p, chunk*e]
    wb = np.ascontiguousarray(
        (w_t[F8CH * P:] * PSUM_SCALE).reshape(NBF, P, E).transpose(1, 0, 2)
        .reshape(P, NBF * E)).astype(bf16)
    swc = np.ascontiguousarray(sw[:, 0].reshape(EC, P).T.astype(bf16))
    w8 = np.ascontiguousarray(
        (w_t[:F8CH * P] * W_SCALE).reshape(F8CH, P, E).transpose(1, 0, 2)
        .reshape(P, F8CH * E)).astype(f8)
    decw_all = dec @ wfc.T                                 # (B, E) fp32

    in_maps = []
    for i in range(NCORES):
        b0 = i * BLOC
        sl = enc[:, b0:b0 + BLOC, :]
        ett = np.ascontiguousarray(sl.transpose(2, 1, 0))  # (D, BLOC, S)
        m = {
            "et8": (ett[:F8CH * P] * ENC_SCALE).astype(f8),
            "etb": ett.astype(bf16),
            "w8": w8,
            "wb": wb,
            "decw": np.ascontiguousarray(
                decw_all[b0:b0 + BLOC].T.reshape(EC, P, BLOC)
                .transpose(1, 0, 2).reshape(P, EC * BLOC)),
            "swc": swc,
            # (BLOC-1, S, DP2): last NP2PE d-chunks, [s, d] layout
            "enp": np.ascontiguousarray(
                sl[:, :BLOC - 1, D - DP2:].transpose(1, 0, 2)).astype(bf16),
            "en3": np.ascontiguousarray(sl[:, BLOC - 1, :]).astype(bf16),
        }
        in_maps.append(m)
    return in_maps


def kernel(encoder_states, decoder_state, W_fc, score_w):
    from concourse.bass_utils import run_bass_kernel_spmd

    in_maps = make_in_maps(encoder_states, decoder_state, W_fc, score_w)
    nc = _get_program()
    res = run_bass_kernel_spmd(nc, in_maps, list(range(NCORES)))
    return np.concatenate([res.results[i]["out"] for i in range(NCORES)], axis=0)


# revision 39
# speedup vs baseline: 1.0033x; 1.0033x over previous
"""Trainium2 Bass kernel for the AttentionLoop module.

Reference computation (S=2048, B=32, D=1024, E=1024):
    h = tanh(einsum('sbd,ed->sbe', dec + enc, W_fc))
    scores = einsum('sbe,e->bs', h, score_w[:,0])
    attn = softmax(scores, axis=1)          # over seq
    out = einsum('bs,sbd->bd', attn, enc)   # (B, D)

Data-parallel over batch across 8 NeuronCores (4 batches/core), core-local.

Per-core kernel, hybrid-precision, h in [e-part, s-free] orientation:
  - pass-1 matmuls use W chunks as stationary, enc chunks as moving:
    out tile = [128 e, 512 s] PSUM accumulated over d-chunks. The first
    2*NPAIR d-chunks run as fp8(e4m3) DoubleRow matmuls; the rest as
    bf16 matmuls (fp8 on all 8 chunks would breach the 2e-2 rel-err
    budget: quantization noise on 6/8 chunks already contributes
    ~1.8e-2). Host pre-scales: enc*4, W*32 -> psum 128x, folded out in
    the tanh evac.
  - decoder bias decW[b,e] = dec @ W.T is precomputed on the HOST in
    fp32 and rides the ScalarE tanh evac as a per-partition bias.
  - scores: the sw-weighted e-reduction is hierarchical. Per e-chunk
    one DVE scalar_tensor_tensor over the full [128, 2048] h tile:
    g += h * sw_col (per-partition scalar) folds multiply + 8-chunk
    accumulation; the final 128-partition reduce is 4 ones-stationary
    matmuls per batch (2048 PE cols/batch instead of 16384).
  - pass-2 out[b] = p @ enc is split to balance DVE vs PE: d-chunks
    0..5 on the DVE (stt with accum_out over the resident [d,s] bf16
    enc tiles), d-chunks 6,7 on the TensorE against a small [s, 256]
    enc copy (en3p), with p transposed to columns via K=1 one-hot
    matmuls of the broadcast p. Both halves hide under the next
    batch's pass-1. The last batch runs pass-2 fully on the TensorE
    (all-DVE would be exposed at the end), via a full [s, d] enc copy.
  - softmax skips max-subtraction (scores are O(1)); Exp activation
    with accum_out gives the denominator partials for free.
  - DMA: two HWDGE queues carry pass-1-critical loads in consumption
    order (sync: w8/etb67/et8, scalar: et80/wb + pass-2 copies); tiny
    out stores ride the GpSimd SWDGE queue so their wait on the
    pass-2 result never stalls the input queues.
"""

import numpy as np

S, B, D, E = 2048, 32, 1024, 1024
NCORES = 8
BLOC = B // NCORES          # 4 batches per core
P = 128                     # partitions
DC = D // P                 # 8 d-chunks
EC = E // P                 # 8 e-chunks
SB = 512                    # moving free dim (PSUM bank)
NSBLK = S // SB             # 4 s-blocks per batch
NSC = S // P                # 16 s-chunks per batch

NPAIR = 3                   # d-chunk pairs done in fp8 DoubleRow
F8CH = 2 * NPAIR            # d-chunks covered by fp8
NBF = DC - F8CH             # bf16 d-chunks
NP2PE = 2                   # pass-2 d-chunks on TensorE (rest on DVE)
NP2DVE = DC - NP2PE
DP2 = NP2PE * P             # pass-2 PE columns
ENC_SCALE = 4.0             # host pre-scale on fp8 enc
W_SCALE = 32.0              # host pre-scale on fp8 W
PSUM_SCALE = ENC_SCALE * W_SCALE   # bf16 W copy is scaled by this too

_compiled = None


def _build_program():
    import concourse.bacc as bacc
    import concourse.mybir as mybir
    import concourse.tile as tile

    f32 = mybir.dt.float32
    bf16 = mybir.dt.bfloat16
    f8 = mybir.dt.float8e4
    AF = mybir.ActivationFunctionType
    DR = mybir.MatmulPerfMode.DoubleRow
    ALU = mybir.AluOpType

    nc = bacc.Bacc("TRN2", target_bir_lowering=False, debug=False,
                   num_devices=NCORES)

    et8_d = nc.declare_dram_parameter("et8", [F8CH * P, BLOC, S], f8,
                                      isOutput=False)
    etb_d = nc.declare_dram_parameter("etb", [D, BLOC, S], bf16, isOutput=False)
    # weight/bias layouts are pre-permuted on the host so each SBUF
    # partition's data is one contiguous DRAM run (128 fat DMA descriptors
    # instead of 256-1024 thin ones -> cheap HWDGE triggers)
    w8_d = nc.declare_dram_parameter("w8", [P, F8CH * E], f8, isOutput=False)
    wb_d = nc.declare_dram_parameter("wb", [P, NBF * E], bf16, isOutput=False)
    decw_d = nc.declare_dram_parameter("decw", [P, EC * BLOC], f32,
                                       isOutput=False)
    swc_d = nc.declare_dram_parameter("swc", [P, EC], bf16, isOutput=False)
    # [s, d] bf16 enc copies for PE-side pass-2: last 2 d-chunks for b<3,
    # full D for the tail batch
    enp_d = nc.declare_dram_parameter("enp", [BLOC - 1, S, DP2], bf16,
                                      isOutput=False)
    en3_d = nc.declare_dram_parameter("en3", [S, D], bf16, isOutput=False)
    out_d = nc.declare_dram_parameter("out", [BLOC, D], f32, isOutput=True)

    with tile.TileContext(nc) as tc:
        with tc.tile_pool(name="const", bufs=1) as const, \
             tc.tile_pool(name="et8", bufs=2) as et8_pool, \
             tc.tile_pool(name="etb", bufs=2) as etb_pool, \
             tc.tile_pool(name="enp", bufs=2) as enp_pool, \
             tc.tile_pool(name="h", bufs=4) as h_pool, \
             tc.tile_pool(name="g", bufs=2) as g_pool, \
             tc.tile_pool(name="pbc", bufs=2) as pbc_pool, \
             tc.tile_pool(name="pcl", bufs=2) as pcl_pool, \
             tc.tile_pool(name="scr", bufs=2) as scr_pool, \
             tc.tile_pool(name="misc", bufs=2) as misc, \
             tc.tile_pool(name="ph", bufs=3, space="PSUM") as ph_pool, \
             tc.tile_pool(name="psc", bufs=2, space="PSUM") as psc_pool, \
             tc.tile_pool(name="pt", bufs=1, space="PSUM") as pt_pool, \
             tc.tile_pool(name="po", bufs=2, space="PSUM") as po_pool:

            etb_r = etb_d.ap().rearrange("(dc p) b s -> p dc b s", p=P)
            wb_r = wb_d.ap().rearrange("p (dc e) -> p dc e", dc=NBF)
            decw_r = decw_d.ap().rearrange("p (ec b) -> p ec b", ec=EC)
            enp_r = enp_d.ap().rearrange("b (sc p) d -> p b sc d", p=P)
            en3_r = en3_d.ap().rearrange("(sc p) d -> p sc d", p=P)
            out_r = out_d.ap().rearrange("b (dc p) -> p b dc", p=P)
            et8_r = et8_d.ap().rearrange("(c p) b s -> p c b s", p=P)
            w8_r = w8_d.ap().rearrange("p (c e) -> p c e", c=F8CH)

            # ---- startup DMAs, critical-first, spread over 3 queues ----
            decw_col = const.tile([P, EC, BLOC], f32)
            swc_sb = const.tile([P, EC], bf16)
            w8_sb = const.tile([P, F8CH, E], f8)
            wb_sb = const.tile([P, NBF, E], bf16)
            et80 = et8_pool.tile([P, F8CH, S], f8, tag="et8", name="et80")
            etb0 = etb_pool.tile([P, DC, S], bf16, tag="etb", name="etb0")

            for c in range(0, F8CH, 2):
                nc.sync.dma_start(w8_sb[:, c:c + 2, :], w8_r[:, c:c + 2, :])
            for c in range(0, F8CH, 2):
                nc.scalar.dma_start(et80[:, c:c + 2, :],
                                    et8_r[:, c:c + 2, 0, :])
            nc.gpsimd.dma_start(etb0[:, F8CH:DC, :], etb_r[:, F8CH:DC, 0, :])
            nc.sync.dma_start(wb_sb[:], wb_r)
            nc.scalar.dma_start(decw_col[:], decw_r)
            nc.scalar.dma_start(swc_sb[:], swc_d.ap())

            ones_col = const.tile([P, 1], bf16)
            nc.vector.memset(ones_col[:], 1.0)
            e0 = const.tile([P, 1], bf16)
            nc.vector.memset(e0[:], 0.0)
            nc.vector.memset(e0[0:1, :], 1.0)

            for b in range(BLOC):
                last = (b == BLOC - 1)
                if b == 0:
                    etb, et8 = etb0, et80
                else:
                    etb = etb_pool.tile([P, DC, S], bf16, tag="etb",
                                        name=f"etb{b}")
                    et8 = et8_pool.tile([P, F8CH, S], f8, tag="et8",
                                        name=f"et8{b}")
                    for c in range(0, F8CH, 2):
                        nc.sync.dma_start(et8[:, c:c + 2, :],
                                          et8_r[:, c:c + 2, b, :])
                    nc.sync.dma_start(etb[:, F8CH:DC, :],
                                      etb_r[:, F8CH:DC, b, :])
                if not last:
                    enp_sb = enp_pool.tile([P, NSC, DP2], bf16, tag="enp",
                                           name=f"enp{b}")
                    nc.scalar.dma_start(enp_sb[:], enp_r[:, b, :, :])
                if b in (1, 2):
                    # tail batch's full [s, d] enc copy, in halves at the
                    # starts of b1 and b2 on the Scalar queue
                    if b == 1:
                        en3_sb = const.tile([P, NSC, D], bf16)
                    q0 = (b - 1) * 8
                    nc.scalar.dma_start(en3_sb[:, q0:q0 + 4, :],
                                        en3_r[:, q0:q0 + 4, :])
                    nc.scalar.dma_start(en3_sb[:, q0 + 4:q0 + 8, :],
                                        en3_r[:, q0 + 4:q0 + 8, :])

                sc_ps = [psc_pool.tile([1, SB], f32, tag="sc",
                                       name=f"sc{j}")
                         for j in range(NSBLK)]
                g_acc = g_pool.tile([P, NSBLK, SB], bf16, tag="g",
                                    name=f"g{b}")

                for ec in range(EC):
                    ecs = slice(ec * P, (ec + 1) * P)
                    # deferred emission of this batch's DVE-pass-2 bf16 enc
                    # chunks on the GpSimd SWDGE queue: separate semaphore
                    # lanes, so these late-completing loads never couple
                    # into pass-1 consumers' conservative DMA waits
                    if not last and ec in (1, 2, 3):
                        dcp = 2 * (ec - 1)
                        nc.scalar.dma_start(etb[:, dcp:dcp + 2, :],
                                            etb_r[:, dcp:dcp + 2, b, :])
                    h_ec = h_pool.tile([P, NSBLK, SB], bf16, tag="h",
                                       name=f"h{ec}")
                    if b == 0 and ec == 0:
                        # cold start: accumulate pair-outer across three
                        # s-blocks (fits the 3-deep psum ring) so the PE
                        # works on whichever operand pairs have landed
                        phs = [ph_pool.tile([P, SB], f32, tag="ph",
                                            name=f"ph00_{j}")
                               for j in range(3)]
                        for pr in range(NPAIR):
                            for sblk in range(3):
                                ss = slice(sblk * SB, (sblk + 1) * SB)
                                nc.tensor.matmul(
                                    phs[sblk][:],
                                    w8_sb[:, 2 * pr:2 * pr + 2, ecs],
                                    et8[:, 2 * pr:2 * pr + 2, ss],
                                    start=(pr == 0), stop=False,
                                    perf_mode=DR)
                        for j in range(NBF):
                            for sblk in range(3):
                                ss = slice(sblk * SB, (sblk + 1) * SB)
                                nc.tensor.matmul(
                                    phs[sblk][:], wb_sb[:, j, ecs],
                                    etb[:, F8CH + j, ss],
                                    start=False, stop=(j == NBF - 1))
                        for sblk in range(3):
                            nc.scalar.activation(
                                h_ec[:, sblk, :], phs[sblk][:], AF.Tanh,
                                bias=decw_col[:, ec, b:b + 1],
                                scale=1.0 / PSUM_SCALE)
                        ph3 = ph_pool.tile([P, SB], f32, tag="ph",
                                           name="ph00_3")
                        for pr in range(NPAIR):
                            nc.tensor.matmul(
                                ph3[:], w8_sb[:, 2 * pr:2 * pr + 2, ecs],
                                et8[:, 2 * pr:2 * pr + 2, 3 * SB:4 * SB],
                                start=(pr == 0), stop=False, perf_mode=DR)
                        for j in range(NBF):
                            nc.tensor.matmul(
                                ph3[:], wb_sb[:, j, ecs],
                                etb[:, F8CH + j, 3 * SB:4 * SB],
                                start=False, stop=(j == NBF - 1))
                        nc.scalar.activation(
                            h_ec[:, 3, :], ph3[:], AF.Tanh,
                            bias=decw_col[:, ec, b:b + 1],
                            scale=1.0 / PSUM_SCALE)
                        nc.vector.scalar_tensor_tensor(
                            g_acc[:], h_ec[:],
                            swc_sb[:, ec:ec + 1], h_ec[:],
                            ALU.mult, ALU.bypass)
                        continue
                    for sblk in range(NSBLK):
                        ss = slice(sblk * SB, (sblk + 1) * SB)
                        # pipeline the previous s-block's partition-reduce
                        # into this matmul stream (PE never waits on DVE)
                        if ec == EC - 1 and sblk >= 1:
                            nc.tensor.matmul(
                                sc_ps[sblk - 1][:], ones_col[:],
                                g_acc[:, sblk - 1, :],
                                start=True, stop=True)
                        ph = ph_pool.tile([P, SB], f32, tag="ph",
                                          name=f"ph{sblk}")
                        for pr in range(NPAIR):
                            nc.tensor.matmul(
                                ph[:],
                                w8_sb[:, 2 * pr:2 * pr + 2, ecs],
                                et8[:, 2 * pr:2 * pr + 2, ss],
                                start=(pr == 0), stop=False,
                                perf_mode=DR)
                        for j in range(NBF):
                            nc.tensor.matmul(
                                ph[:], wb_sb[:, j, ecs],
                                etb[:, F8CH + j, ss],
                                start=False, stop=(j == NBF - 1))
                        nc.scalar.activation(
                            h_ec[:, sblk, :], ph[:], AF.Tanh,
                            bias=decw_col[:, ec, b:b + 1],
                            scale=1.0 / PSUM_SCALE)
                        # the last e-chunk's accumulation runs per-s-block so
                        # the pipelined partition-reduce matmuls above see
                        # fully-accumulated g for s-blocks 0..2
                        if ec == EC - 1:
                            nc.vector.scalar_tensor_tensor(
                                g_acc[:, sblk, :], h_ec[:, sblk, :],
                                swc_sb[:, ec:ec + 1], g_acc[:, sblk, :],
                                ALU.mult, ALU.add)
                    # one DVE op per e-chunk folds the sw multiply and the
                    # chunk accumulation over the whole [128, 2048] tile
                    if ec == 0:
                        nc.vector.scalar_tensor_tensor(
                            g_acc[:], h_ec[:],
                            swc_sb[:, ec:ec + 1], h_ec[:],
                            ALU.mult, ALU.bypass)
                    elif ec < EC - 1:
                        nc.vector.scalar_tensor_tensor(
                            g_acc[:], h_ec[:],
                            swc_sb[:, ec:ec + 1], g_acc[:],
                            ALU.mult, ALU.add)

                # last s-block's partition-reduce (0..2 were pipelined)
                nc.tensor.matmul(
                    sc_ps[NSBLK - 1][:], ones_col[:],
                    g_acc[:, NSBLK - 1, :], start=True, stop=True)

                # ---- softmax (no max-subtraction; scores are O(1)) ----
                p_row = misc.tile([1, S], bf16, tag="p")
                lp = misc.tile([1, NSBLK], f32, tag="lp")
                p_bc = pbc_pool.tile([P, S], bf16, tag="pbc")
                pcol = pcl_pool.tile([P, NSC], bf16, tag="pcol")
                pct_all = pt_pool.tile([P, NSC], f32, tag="pt")
                npsb = NSC // NSBLK  # p-columns per s-block
                en_pe = en3_sb if last else enp_sb
                wid = D if last else DP2
                if last:
                    # tail: pass-1 is done, reuse the ph psum ring
                    po = [ph_pool.tile([1, SB], f32, tag="ph",
                                       name=f"po{b}_{g}")
                          for g in range(D // SB)]
                else:
                    po = [po_pool.tile([1, DP2], f32, tag="po",
                                       name=f"po{b}_0")]
                for sblk in range(NSBLK):
                    ss = slice(sblk * SB, (sblk + 1) * SB)
                    nc.scalar.activation(
                        p_row[:, ss], sc_ps[sblk][:],
                        AF.Exp, accum_out=lp[:, sblk:sblk + 1])
                    nc.gpsimd.partition_broadcast(p_bc[:, ss], p_row[:, ss])
                    for k in range(npsb):
                        sc = sblk * npsb + k
                        nc.tensor.matmul(
                            pct_all[:, sc:sc + 1],
                            p_bc[:, sc * P:(sc + 1) * P], e0[:],
                            start=True, stop=True,
                            skip_group_check=True)
                    nc.scalar.activation(
                        pcol[:, sblk * npsb:(sblk + 1) * npsb],
                        pct_all[:, sblk * npsb:(sblk + 1) * npsb], AF.Copy)
                    # PE-side pass-2 for this s-block's p-columns
                    for k in range(npsb):
                        sc = sblk * npsb + k
                        for g in range(max(1, wid // SB)):
                            gw = min(SB, wid)
                            nc.tensor.matmul(
                                po[g][:], pcol[:, sc:sc + 1],
                                en_pe[:, sc, g * gw:(g + 1) * gw],
                                start=(sc == 0), stop=(sc == NSC - 1))

                lt = misc.tile([1, 1], f32, tag="lt")
                nc.vector.tensor_reduce(lt[:], lp[:], mybir.AxisListType.X,
                                        mybir.AluOpType.add)
                invl = misc.tile([1, 1], f32, tag="invl")
                nc.vector.reciprocal(invl[:], lt[:])

                if not last:
                    # ---- DVE pass-2 for d-chunks 0..NP2DVE-1 ----
                    invl_bc = misc.tile([P, 1], f32, tag="invlbc")
                    nc.gpsimd.partition_broadcast(invl_bc[:], invl[:])
                    ocol = misc.tile([P, NP2DVE], f32, tag="ocol")
                    scr = scr_pool.tile([P, S], bf16, tag="scr")
                    for dc in range(NP2DVE):
                        nc.vector.scalar_tensor_tensor(
                            scr[:], etb[:, dc, :], 1.0, p_bc[:],
                            ALU.mult, ALU.mult,
                            accum_out=ocol[:, dc:dc + 1])
                    outf = misc.tile([P, NP2DVE], f32, tag="outf")
                    nc.scalar.activation(outf[:], ocol[:], AF.Copy,
                                         scale=invl_bc[:])
                    nc.gpsimd.dma_start(out_r[:, b, 0:NP2DVE], outf[:])
                    # PE-side chunks: scale + store the [1, DP2] row
                    orow = misc.tile([1, DP2], f32, tag="orow")
                    nc.scalar.activation(orow[:], po[0][:], AF.Copy,
                                         scale=invl[:])
                    nc.gpsimd.dma_start(
                        out_d.ap()[b:b + 1, D - DP2:D], orow[:])
                else:
                    out_row = misc.tile([1, D], f32, tag="orow3")
                    for g in range(D // SB):
                        nc.scalar.activation(
                            out_row[:, g * SB:(g + 1) * SB], po[g][:],
                            AF.Copy, scale=invl[:])
                    nc.sync.dma_start(out_d.ap()[b:b + 1, :], out_row[:])

    nc.compile()
    return nc


def _get_program():
    global _compiled
    if _compiled is None:
        _compiled = _build_program()
    return _compiled


def make_in_maps(encoder_states, decoder_state, W_fc, score_w):
    """Shard + lay out + cast full inputs into per-core input maps."""
    import ml_dtypes
    f8 = ml_dtypes.float8_e4m3
    bf16 = ml_dtypes.bfloat16

    enc = np.asarray(encoder_states, dtype=np.float32)
    dec = np.asarray(decoder_state, dtype=np.float32)
    wfc = np.asarray(W_fc, dtype=np.float32)
    sw = np.asarray(score_w, dtype=np.float32)

    w_t = np.ascontiguousarray(wfc.T)                      # (D, E)
    # per-partition-contiguous permutations: [chunk, p, e] -> [p, chunk*e]
    wb = np.ascontiguousarray(
        (w_t[F8CH * P:] * PSUM_SCALE).reshape(NBF, P, E).transpose(1, 0, 2)
        .reshape(P, NBF * E)).astype(bf16)
    swc = np.ascontiguousarray(sw[:, 0].reshape(EC, P).T.astype(bf16))
    w8 = np.ascontiguousarray(
        (w_t[:F8CH * P] * W_SCALE).reshape(F8CH, P, E).transpose(1, 0, 2)
        .reshape(P, F8CH * E)).astype(f8)
    decw_all = dec @ wfc.T                                 # (B, E) fp32

    in_maps = []
    for i in range(NCORES):
        b0 = i * BLOC
        sl = enc[:, b0:b0 + BLOC, :]
        ett = np.ascontiguousarray(sl.transpose(2, 1, 0))  # (D, BLOC, S)
        m = {
            "et8": (ett[:F8CH * P] * ENC_SCALE).astype(f8),
            "etb": ett.astype(bf16),
            "w8": w8,
            "wb": wb,
            "decw": np.ascontiguousarray(
                decw_all[b0:b0 + BLOC].T.reshape(EC, P, BLOC)
                .transpose(1, 0, 2).reshape(P, EC * BLOC)),
            "swc": swc,
            # (BLOC-1, S, DP2): last NP2PE d-chunks, [s, d] layout
            "enp": np.ascontiguousarray(
                sl[:, :BLOC - 1, D - DP2:].transpose(1, 0, 2)).astype(bf16),
            "en3": np.ascontiguousarray(sl[:, BLOC - 1, :]).astype(bf16),
        }
        in_maps.append(m)
    return in_maps


def kernel(encoder_states, decoder_state, W_fc, score_w):
    from concourse.bass_utils import run_bass_kernel_spmd

    in_maps = make_in_maps(encoder_states, decoder_state, W_fc, score_w)
    nc = _get_program()
    res = run_bass_kernel_spmd(nc, in_maps, list(range(NCORES)))
    return np.concatenate([res.results[i]["out"] for i in range(NCORES)], axis=0)


# revision 40
# speedup vs baseline: 1.0051x; 1.0017x over previous
"""Trainium2 Bass kernel for the AttentionLoop module.

Reference computation (S=2048, B=32, D=1024, E=1024):
    h = tanh(einsum('sbd,ed->sbe', dec + enc, W_fc))
    scores = einsum('sbe,e->bs', h, score_w[:,0])
    attn = softmax(scores, axis=1)          # over seq
    out = einsum('bs,sbd->bd', attn, enc)   # (B, D)

Data-parallel over batch across 8 NeuronCores (4 batches/core), core-local.

Per-core kernel, hybrid-precision, h in [e-part, s-free] orientation:
  - pass-1 matmuls use W chunks as stationary, enc chunks as moving:
    out tile = [128 e, 512 s] PSUM accumulated over d-chunks. The first
    2*NPAIR d-chunks run as fp8(e4m3) DoubleRow matmuls; the rest as
    bf16 matmuls (fp8 on all 8 chunks would breach the 2e-2 rel-err
    budget: quantization noise on 6/8 chunks already contributes
    ~1.8e-2). Host pre-scales: enc*4, W*32 -> psum 128x, folded out in
    the tanh evac.
  - decoder bias decW[b,e] = dec @ W.T is precomputed on the HOST in
    fp32 and rides the ScalarE tanh evac as a per-partition bias.
  - scores: the sw-weighted e-reduction is hierarchical. Per e-chunk
    one DVE scalar_tensor_tensor over the full [128, 2048] h tile:
    g += h * sw_col (per-partition scalar) folds multiply + 8-chunk
    accumulation; the final 128-partition reduce is 4 ones-stationary
    matmuls per batch (2048 PE cols/batch instead of 16384).
  - pass-2 out[b] = p @ enc is split to balance DVE vs PE: d-chunks
    0..5 on the DVE (stt with accum_out over the resident [d,s] bf16
    enc tiles), d-chunks 6,7 on the TensorE against a small [s, 256]
    enc copy (en3p), with p transposed to columns via K=1 one-hot
    matmuls of the broadcast p. Both halves hide under the next
    batch's pass-1. The last batch runs pass-2 fully on the TensorE
    (all-DVE would be exposed at the end), via a full [s, d] enc copy.
  - softmax skips max-subtraction (scores are O(1)); Exp activation
    with accum_out gives the denominator partials for free.
  - DMA: two HWDGE queues carry pass-1-critical loads in consumption
    order (sync: w8/etb67/et8, scalar: et80/wb + pass-2 copies); tiny
    out stores ride the GpSimd SWDGE queue so their wait on the
    pass-2 result never stalls the input queues.
"""

import numpy as np

S, B, D, E = 2048, 32, 1024, 1024
NCORES = 8
BLOC = B // NCORES          # 4 batches per core
P = 128                     # partitions
DC = D // P                 # 8 d-chunks
EC = E // P                 # 8 e-chunks
SB = 512                    # moving free dim (PSUM bank)
NSBLK = S // SB             # 4 s-blocks per batch
NSC = S // P                # 16 s-chunks per batch

NPAIR = 3                   # d-chunk pairs done in fp8 DoubleRow
F8CH = 2 * NPAIR            # d-chunks covered by fp8
NBF = DC - F8CH             # bf16 d-chunks
NP2PE = 2                   # pass-2 d-chunks on TensorE (rest on DVE)
NP2DVE = DC - NP2PE
DP2 = NP2PE * P             # pass-2 PE columns
ENC_SCALE = 4.0             # host pre-scale on fp8 enc
W_SCALE = 32.0              # host pre-scale on fp8 W
PSUM_SCALE = ENC_SCALE * W_SCALE   # bf16 W copy is scaled by this too

_compiled = None


def _build_program():
    import concourse.bacc as bacc
    import concourse.mybir as mybir
    import concourse.tile as tile

    f32 = mybir.dt.float32
    bf16 = mybir.dt.bfloat16
    f8 = mybir.dt.float8e4
    AF = mybir.ActivationFunctionType
    DR = mybir.MatmulPerfMode.DoubleRow
    ALU = mybir.AluOpType

    nc = bacc.Bacc("TRN2", target_bir_lowering=False, debug=False,
                   num_devices=NCORES)

    et8_d = nc.declare_dram_parameter("et8", [F8CH * P, BLOC, S], f8,
                                      isOutput=False)
    etb_d = nc.declare_dram_parameter("etb", [D, BLOC, S], bf16, isOutput=False)
    # weight/bias layouts are pre-permuted on the host so each SBUF
    # partition's data is one contiguous DRAM run (128 fat DMA descriptors
    # instead of 256-1024 thin ones -> cheap HWDGE triggers)
    w8_d = nc.declare_dram_parameter("w8", [P, F8CH * E], f8, isOutput=False)
    wb_d = nc.declare_dram_parameter("wb", [P, NBF * E], bf16, isOutput=False)
    decw_d = nc.declare_dram_parameter("decw", [P, EC * BLOC], f32,
                                       isOutput=False)
    swc_d = nc.declare_dram_parameter("swc", [P, EC], bf16, isOutput=False)
    # [s, d] bf16 enc copies for PE-side pass-2: last 2 d-chunks for b<3,
    # full D for the tail batch
    enp_d = nc.declare_dram_parameter("enp", [BLOC - 1, S, DP2], bf16,
                                      isOutput=False)
    en3_d = nc.declare_dram_parameter("en3", [S, D], bf16, isOutput=False)
    out_d = nc.declare_dram_parameter("out", [BLOC, D], f32, isOutput=True)

    with tile.TileContext(nc) as tc:
        with tc.tile_pool(name="const", bufs=1) as const, \
             tc.tile_pool(name="et8", bufs=2) as et8_pool, \
             tc.tile_pool(name="etb", bufs=2) as etb_pool, \
             tc.tile_pool(name="enp", bufs=2) as enp_pool, \
             tc.tile_pool(name="h", bufs=4) as h_pool, \
             tc.tile_pool(name="g", bufs=2) as g_pool, \
             tc.tile_pool(name="pbc", bufs=2) as pbc_pool, \
             tc.tile_pool(name="pcl", bufs=2) as pcl_pool, \
             tc.tile_pool(name="scr", bufs=2) as scr_pool, \
             tc.tile_pool(name="misc", bufs=2) as misc, \
             tc.tile_pool(name="ph", bufs=3, space="PSUM") as ph_pool, \
             tc.tile_pool(name="psc", bufs=2, space="PSUM") as psc_pool, \
             tc.tile_pool(name="pt", bufs=1, space="PSUM") as pt_pool, \
             tc.tile_pool(name="po", bufs=2, space="PSUM") as po_pool:

            etb_r = etb_d.ap().rearrange("(dc p) b s -> p dc b s", p=P)
            wb_r = wb_d.ap().rearrange("p (dc e) -> p dc e", dc=NBF)
            decw_r = decw_d.ap().rearrange("p (ec b) -> p ec b", ec=EC)
            enp_r = enp_d.ap().rearrange("b (sc p) d -> p b sc d", p=P)
            en3_r = en3_d.ap().rearrange("(sc p) d -> p sc d", p=P)
            out_r = out_d.ap().rearrange("b (dc p) -> p b dc", p=P)
            et8_r = et8_d.ap().rearrange("(c p) b s -> p c b s", p=P)
            w8_r = w8_d.ap().rearrange("p (c e) -> p c e", c=F8CH)

            # ---- startup DMAs, critical-first, spread over 3 queues ----
            decw_col = const.tile([P, EC, BLOC], f32)
            swc_sb = const.tile([P, EC], bf16)
            w8_sb = const.tile([P, F8CH, E], f8)
            wb_sb = const.tile([P, NBF, E], bf16)
            et80 = et8_pool.tile([P, F8CH, S], f8, tag="et8", name="et80")
            etb0 = etb_pool.tile([P, DC, S], bf16, tag="etb", name="etb0")

            for c in range(0, F8CH, 2):
                nc.sync.dma_start(w8_sb[:, c:c + 2, :], w8_r[:, c:c + 2, :])
            for c in range(0, F8CH, 2):
                nc.scalar.dma_start(et80[:, c:c + 2, :],
                                    et8_r[:, c:c + 2, 0, :])
            nc.gpsimd.dma_start(etb0[:, F8CH:DC, :], etb_r[:, F8CH:DC, 0, :])
            nc.sync.dma_start(wb_sb[:], wb_r)
            nc.scalar.dma_start(decw_col[:], decw_r)
            nc.scalar.dma_start(swc_sb[:], swc_d.ap())

            ones_col = const.tile([P, 1], bf16)
            nc.vector.memset(ones_col[:], 1.0)
            e0 = const.tile([P, 1], bf16)
            nc.vector.memset(e0[:], 0.0)
            nc.vector.memset(e0[0:1, :], 1.0)

            for b in range(BLOC):
                last = (b == BLOC - 1)
                if b == 0:
                    etb, et8 = etb0, et80
                else:
                    etb = etb_pool.tile([P, DC, S], bf16, tag="etb",
                                        name=f"etb{b}")
                    et8 = et8_pool.tile([P, F8CH, S], f8, tag="et8",
                                        name=f"et8{b}")
                    for c in range(0, F8CH, 2):
                        nc.sync.dma_start(et8[:, c:c + 2, :],
                                          et8_r[:, c:c + 2, b, :])
                    nc.sync.dma_start(etb[:, F8CH:DC, :],
                                      etb_r[:, F8CH:DC, b, :])
                if not last:
                    enp_sb = enp_pool.tile([P, NSC, DP2], bf16, tag="enp",
                                           name=f"enp{b}")
                    nc.scalar.dma_start(enp_sb[:], enp_r[:, b, :, :])
                if b in (1, 2):
                    # tail batch's full [s, d] enc copy, in halves at the
                    # starts of b1 and b2 on the Scalar queue
                    if b == 1:
                        en3_sb = const.tile([P, NSC, D], bf16)
                    q0 = (b - 1) * 8
                    nc.scalar.dma_start(en3_sb[:, q0:q0 + 4, :],
                                        en3_r[:, q0:q0 + 4, :])
                    nc.scalar.dma_start(en3_sb[:, q0 + 4:q0 + 8, :],
                                        en3_r[:, q0 + 4:q0 + 8, :])

                sc_ps = [psc_pool.tile([1, SB], f32, tag="sc",
                                       name=f"sc{j}")
                         for j in range(NSBLK)]
                g_acc = g_pool.tile([P, NSBLK, SB], bf16, tag="g",
                                    name=f"g{b}")

                for ec in range(EC):
                    ecs = slice(ec * P, (ec + 1) * P)
                    # deferred emission of this batch's DVE-pass-2 bf16 enc
                    # chunks on the GpSimd SWDGE queue: separate semaphore
                    # lanes, so these late-completing loads never couple
                    # into pass-1 consumers' conservative DMA waits
                    if not last and ec in (1, 2, 3):
                        dcp = 2 * (ec - 1)
                        nc.scalar.dma_start(etb[:, dcp:dcp + 2, :],
                                            etb_r[:, dcp:dcp + 2, b, :])
                    h_ec = h_pool.tile([P, NSBLK, SB], bf16, tag="h",
                                       name=f"h{ec}")
                    if b == 0 and ec == 0:
                        # cold start: accumulate pair-outer across three
                        # s-blocks (fits the 3-deep psum ring) so the PE
                        # works on whichever operand pairs have landed
                        phs = [ph_pool.tile([P, SB], f32, tag="ph",
                                            name=f"ph00_{j}")
                               for j in range(3)]
                        for pr in range(NPAIR):
                            for sblk in range(3):
                                ss = slice(sblk * SB, (sblk + 1) * SB)
                                nc.tensor.matmul(
                                    phs[sblk][:],
                                    w8_sb[:, 2 * pr:2 * pr + 2, ecs],
                                    et8[:, 2 * pr:2 * pr + 2, ss],
                                    start=(pr == 0), stop=False,
                                    perf_mode=DR)
                        for j in range(NBF):
                            for sblk in range(3):
                                ss = slice(sblk * SB, (sblk + 1) * SB)
                                nc.tensor.matmul(
                                    phs[sblk][:], wb_sb[:, j, ecs],
                                    etb[:, F8CH + j, ss],
                                    start=False, stop=(j == NBF - 1))
                        for sblk in range(3):
                            nc.scalar.activation(
                                h_ec[:, sblk, :], phs[sblk][:], AF.Tanh,
                                bias=decw_col[:, ec, b:b + 1],
                                scale=1.0 / PSUM_SCALE)
                        ph3 = ph_pool.tile([P, SB], f32, tag="ph",
                                           name="ph00_3")
                        for pr in range(NPAIR):
                            nc.tensor.matmul(
                                ph3[:], w8_sb[:, 2 * pr:2 * pr + 2, ecs],
                                et8[:, 2 * pr:2 * pr + 2, 3 * SB:4 * SB],
                                start=(pr == 0), stop=False, perf_mode=DR)
                        for j in range(NBF):
                            nc.tensor.matmul(
                                ph3[:], wb_sb[:, j, ecs],
                                etb[:, F8CH + j, 3 * SB:4 * SB],
                                start=False, stop=(j == NBF - 1))
                        nc.scalar.activation(
                            h_ec[:, 3, :], ph3[:], AF.Tanh,
                            bias=decw_col[:, ec, b:b + 1],
                            scale=1.0 / PSUM_SCALE)
                        nc.vector.scalar_tensor_tensor(
                            g_acc[:], h_ec[:],
                            swc_sb[:, ec:ec + 1], h_ec[:],
                            ALU.mult, ALU.bypass)
                        continue
                    for sblk in range(NSBLK):
                        ss = slice(sblk * SB, (sblk + 1) * SB)
                        # pipeline the previous s-block's partition-reduce
                        # into this matmul stream (PE never waits on DVE)
                        if ec == EC - 1 and sblk >= 1:
                            nc.tensor.matmul(
                                sc_ps[sblk - 1][:], ones_col[:],
                                g_acc[:, sblk - 1, :],
                                start=True, stop=True)
                        ph = ph_pool.tile([P, SB], f32, tag="ph",
                                          name=f"ph{sblk}")
                        for pr in range(NPAIR):
                            nc.tensor.matmul(
                                ph[:],
                                w8_sb[:, 2 * pr:2 * pr + 2, ecs],
                                et8[:, 2 * pr:2 * pr + 2, ss],
                                start=(pr == 0), stop=False,
                                perf_mode=DR)
                        for j in range(NBF):
                            nc.tensor.matmul(
                                ph[:], wb_sb[:, j, ecs],
                                etb[:, F8CH + j, ss],
                                start=False, stop=(j == NBF - 1))
                        nc.scalar.activation(
                            h_ec[:, sblk, :], ph[:], AF.Tanh,
                            bias=decw_col[:, ec, b:b + 1],
                            scale=1.0 / PSUM_SCALE)
                        # the last e-chunk's accumulation runs per-s-block so
                        # the pipelined partition-reduce matmuls above see
                        # fully-accumulated g for s-blocks 0..2
                        if ec == EC - 1:
                            nc.vector.scalar_tensor_tensor(
                                g_acc[:, sblk, :], h_ec[:, sblk, :],
                                swc_sb[:, ec:ec + 1], g_acc[:, sblk, :],
                                ALU.mult, ALU.add)
                    # one DVE op per e-chunk folds the sw multiply and the
                    # chunk accumulation over the whole [128, 2048] tile
                    if ec == 0:
                        nc.vector.scalar_tensor_tensor(
                            g_acc[:], h_ec[:],
                            swc_sb[:, ec:ec + 1], h_ec[:],
                            ALU.mult, ALU.bypass)
                    elif ec < EC - 1:
                        nc.vector.scalar_tensor_tensor(
                            g_acc[:], h_ec[:],
                            swc_sb[:, ec:ec + 1], g_acc[:],
                            ALU.mult, ALU.add)

                # last s-block's partition-reduce (0..2 were pipelined)
                nc.tensor.matmul(
                    sc_ps[NSBLK - 1][:], ones_col[:],
                    g_acc[:, NSBLK - 1, :], start=True, stop=True)

                # ---- softmax (no max-subtraction; scores are O(1)) ----
                p_row = misc.tile([1, S], bf16, tag="p")
                lp = misc.tile([1, NSBLK], f32, tag="lp")
                p_bc = pbc_pool.tile([P, S], bf16, tag="pbc")
                pcol = pcl_pool.tile([P, NSC], bf16, tag="pcol")
                pct_all = pt_pool.tile([P, NSC], f32, tag="pt")
                npsb = NSC // NSBLK  # p-columns per s-block
                en_pe = en3_sb if last else enp_sb
                wid = D if last else DP2
                if last:
                    # tail: pass-1 is done, reuse the ph psum ring
                    po = [ph_pool.tile([1, SB], f32, tag="ph",
                                       name=f"po{b}_{g}")
                          for g in range(D // SB)]
                else:
                    po = [po_pool.tile([1, DP2], f32, tag="po",
                                       name=f"po{b}_0")]
                for sblk in range(NSBLK):
                    ss = slice(sblk * SB, (sblk + 1) * SB)
                    nc.scalar.activation(
                        p_row[:, ss], sc_ps[sblk][:],
                        AF.Exp, accum_out=lp[:, sblk:sblk + 1])
                    nc.gpsimd.partition_broadcast(p_bc[:, ss], p_row[:, ss])
                    for k in range(npsb):
                        sc = sblk * npsb + k
                        nc.tensor.matmul(
                            pct_all[:, sc:sc + 1],
                            p_bc[:, sc * P:(sc + 1) * P], e0[:],
                            start=True, stop=True,
                            skip_group_check=True)
                    nc.scalar.activation(
                        pcol[:, sblk * npsb:(sblk + 1) * npsb],
                        pct_all[:, sblk * npsb:(sblk + 1) * npsb], AF.Copy)
                    # PE-side pass-2 for this s-block's p-columns
                    for k in range(npsb):
                        sc = sblk * npsb + k
                        for g in range(max(1, wid // SB)):
                            gw = min(SB, wid)
                            nc.tensor.matmul(
                                po[g][:], pcol[:, sc:sc + 1],
                                en_pe[:, sc, g * gw:(g + 1) * gw],
                                start=(sc == 0), stop=(sc == NSC - 1))

                lt = misc.tile([1, 1], f32, tag="lt")
                nc.vector.tensor_reduce(lt[:], lp[:], mybir.AxisListType.X,
                                        mybir.AluOpType.add)
                invl = misc.tile([1, 1], f32, tag="invl")
                nc.vector.reciprocal(invl[:], lt[:])

                if not last:
                    # ---- DVE pass-2 for d-chunks 0..NP2DVE-1 ----
                    invl_bc = misc.tile([P, 1], f32, tag="invlbc")
                    nc.gpsimd.partition_broadcast(invl_bc[:], invl[:])
                    ocol = misc.tile([P, NP2DVE], f32, tag="ocol")
                    scr = scr_pool.tile([P, S], bf16, tag="scr")
                    for dc in range(NP2DVE):
                        nc.vector.scalar_tensor_tensor(
                            scr[:], etb[:, dc, :], 1.0, p_bc[:],
                            ALU.mult, ALU.mult,
                            accum_out=ocol[:, dc:dc + 1])
                    outf = misc.tile([P, NP2DVE], f32, tag="outf")
                    nc.scalar.activation(outf[:], ocol[:], AF.Copy,
                                         scale=invl_bc[:])
                    nc.gpsimd.dma_start(out_r[:, b, 0:NP2DVE], outf[:])
                    # PE-side chunks: scale + store the [1, DP2] row
                    orow = misc.tile([1, DP2], f32, tag="orow")
                    nc.scalar.activation(orow[:], po[0][:], AF.Copy,
                                         scale=invl[:])
                    nc.gpsimd.dma_start(
                        out_d.ap()[b:b + 1, D - DP2:D], orow[:])
                else:
                    # split the final store so the first half's DMA (and its
                    # HBM completion round-trip) overlaps the second scale
                    out_row = misc.tile([1, D], f32, tag="orow3")
                    for g in range(D // SB):
                        nc.scalar.activation(
                            out_row[:, g * SB:(g + 1) * SB], po[g][:],
                            AF.Copy, scale=invl[:])
                        nc.sync.dma_start(
                            out_d.ap()[b:b + 1, g * SB:(g + 1) * SB],
                            out_row[:, g * SB:(g + 1) * SB])

    nc.compile()
    return nc


def _get_program():
    global _compiled
    if _compiled is None:
        _compiled = _build_program()
    return _compiled


def make_in_maps(encoder_states, decoder_state, W_fc, score_w):
    """Shard + lay out + cast full inputs into per-core input maps."""
    import ml_dtypes
    f8 = ml_dtypes.float8_e4m3
    bf16 = ml_dtypes.bfloat16

    enc = np.asarray(encoder_states, dtype=np.float32)
    dec = np.asarray(decoder_state, dtype=np.float32)
    wfc = np.asarray(W_fc, dtype=np.float32)
    sw = np.asarray(score_w, dtype=np.float32)

    w_t = np.ascontiguousarray(wfc.T)                      # (D, E)
    # per-partition-contiguous permutations: [chunk, p, e] -> [p, chunk*e]
    wb = np.ascontiguousarray(
        (w_t[F8CH * P:] * PSUM_SCALE).reshape(NBF, P, E).transpose(1, 0, 2)
        .reshape(P, NBF * E)).astype(bf16)
    swc = np.ascontiguousarray(sw[:, 0].reshape(EC, P).T.astype(bf16))
    w8 = np.ascontiguousarray(
        (w_t[:F8CH * P] * W_SCALE).reshape(F8CH, P, E).transpose(1, 0, 2)
        .reshape(P, F8CH * E)).astype(f8)
    decw_all = dec @ wfc.T                                 # (B, E) fp32

    in_maps = []
    for i in range(NCORES):
        b0 = i * BLOC
        sl = enc[:, b0:b0 + BLOC, :]
        ett = np.ascontiguousarray(sl.transpose(2, 1, 0))  # (D, BLOC, S)
        m = {
            "et8": (ett[:F8CH * P] * ENC_SCALE).astype(f8),
            "etb": ett.astype(bf16),
            "w8": w8,
            "wb": wb,
            "decw": np.ascontiguousarray(
                decw_all[b0:b0 + BLOC].T.reshape(EC, P, BLOC)
                .transpose(1, 0, 2).reshape(P, EC * BLOC)),
            "swc": swc,
            # (BLOC-1, S, DP2): last NP2PE d-chunks, [s, d] layout
            "enp": np.ascontiguousarray(
                sl[:, :BLOC - 1, D - DP2:].transpose(1, 0, 2)).astype(bf16),
            "en3": np.ascontiguousarray(sl[:, BLOC - 1, :]).astype(bf16),
        }
        in_maps.append(m)
    return in_maps


def kernel(encoder_states, decoder_state, W_fc, score_w):
    from concourse.bass_utils import run_bass_kernel_spmd

    in_maps = make_in_maps(encoder_states, decoder_state, W_fc, score_w)
    nc = _get_program()
    res = run_bass_kernel_spmd(nc, in_maps, list(range(NCORES)))
    return np.concatenate([res.results[i]["out"] for i in range(NCORES)], axis=0)
